# revision 15
# baseline (speedup 1.0000x reference)
"""GAT message-passing kernel for Trainium2 (8 NeuronCores, data-parallel over batch).

Math (per batch element b, derived from the reference nn.Module):
    x      = nodes.reshape(N, D)
    self_e = mlp2(x, self_*)                 # [N, H]
    nb_e   = mlp2(x, nb_*)                   # [N, H]
    U      = self_e @ comb_w1[:H] + b1c      # [N, H]  (i side)
    V      = nb_e @ comb_w1[H:]              # [N, H]  (j side)
    scores(i,j) = leaky(U_i + V_j) @ w2 + const_i
                = 0.8*relu(U_i+V_j)@w2 + 0.2*w2.V_j + const_i
    Softmax over j drops per-i constants, so
      X^T[j,i] = exp(0.8 relu(U_i+V_j)@w2 + 0.2 w2.V_j - 30*(1-mask[j,i]))
    where mask[j,i] = edges[j,i]*(j!=i); the -30 bias (host-precomputed
    [N,N] bf16 tile, injected into PSUM by one identity matmul per j-tile)
    replaces all explicit mask multiplies: masked entries come out ~e-13.
    denom[i] = sum_j X^T[j,i] lands in the aggregation matmul via a ones
    column appended to nb_e. (|scores| < 2.5, so exp needs no max-sub.)
    out[i] = gate*(recip * X^T(:,i).nb_e + self_e), gate = denom > 1e-6.

Device mapping (one core per batch element, j-major pairwise stage):
  - Transposed (g,h)-on-partitions layout: partitions = (j-parity g, h), so one
    tensor_scalar(add,max) builds relu(U + V_j) for TWO j's at once as a
    [128, 512] tile over all i. Scores come out j-on-partitions, which is the
    exact layout exp/mask/aggregation need - no PE transposes anywhere.
  - PE reduces over (g,h) two ways, mixed freely inside one 128-row PSUM
    accumulation group per j-tile:
      * bf16 "pair" slots: [128,512] bf16 builds (DVE 4x mode, 194ns) with a
        2-column sliding-window block-diagonal 0.8*w2 lhsT (512cy/matmul);
      * fp8e4 DoubleRow "quad" slots: [128,2,512] fp8 builds (two j-pairs) with
        a [128,2,128] sliding-window lhsT, K=256 contraction at 0.5 cyc/row
        (256cy/matmul = 4 j's) - 4x the bf16 PE throughput.
    Slot kinds interleave (P,Q,P,Q,...) so PE consumption matches the
    aggregate build rate of DVE/ACT/Pool (pattern env-tunable).
  - exp runs on ACT straight out of PSUM with bias = 0.2*w2.V_j (per-partition,
    computed by 4 tiny K=64 matmuls), output bf16 X^T[j,i] used directly as
    the stationary operand of the aggregation matmuls.
  - Aggregation rhs nb_e carries an appended ones column: out[i,64] = denom_i
    lands per-partition in PSUM; gate/recip/assembly are a few [128,64] DVE ops.
  - MLP/U/V precompute runs float32r (full fp32 data, 1 cyc/row at free>=256);
    self_e stays fp32 end-to-end (it adds directly into the output).
"""

import os
import sys

sys.path.insert(0, "/opt/trn_rl_repo")

import numpy as np
import ml_dtypes

import concourse.bass as bass
import concourse.bacc as bacc
import concourse.tile as tile
from concourse import mybir, bass2jax
from concourse.bass_utils import run_bass_kernel_spmd

B, N, H, D = 8, 512, 64, 128
NCORES = 8
NT = N // 128          # 4 j/i tiles of 128
F32 = mybir.dt.float32
F32R = mybir.dt.float32r
BF16 = mybir.dt.bfloat16
FP8 = mybir.dt.float8e4
NP_FP8 = mybir.dt.np(FP8)

# Per-j-tile slot plan: P bf16 pairs (2 rows each) + Q fp8 DoubleRow quads
# (4 rows each), 2P + 4Q = 128.
NPAIRS = int(os.environ.get("GAT_P", "26"))
NQUADS = (128 - 2 * NPAIRS) // 4
assert 2 * NPAIRS + 4 * NQUADS == 128

# Build-engine schedule: 'v' = VectorE, 'a' = ScalarE, 'p' = gpsimd/Pool.
PAIR_PAT = os.environ.get("GAT_PAIR_PAT", "v" * NPAIRS)
QUAD_PAT = os.environ.get("GAT_QUAD_PAT", "")


def _smooth_seq(counts):
    """Smooth weighted round-robin sequence over engine keys."""
    total = sum(counts.values())
    acc = {k: 0.0 for k in counts}
    seq = []
    rem = dict(counts)
    for _ in range(total):
        for k in counts:
            acc[k] += counts[k] / total
        k = max((kk for kk in counts if rem[kk] > 0), key=lambda kk: acc[kk])
        acc[k] -= 1.0
        rem[k] -= 1
        seq.append(k)
    return "".join(seq)


if not QUAD_PAT:
    nq2 = 2 * NQUADS * NT          # consumed globally across all j-tiles
    np_ = (nq2 * 72 + 76) // 152
    na = (nq2 * 47 + 76) // 152
    nv = max(nq2 - np_ - na, 0)
    QUAD_PAT = _smooth_seq({"p": np_, "a": na, "v": nv} if nv else {"p": np_, "a": na})


def _slot_plan():
    """Interleaved (kind, local_j0) slots covering 128 rows."""
    kinds = _smooth_seq({"P": NPAIRS, "Q": NQUADS})
    slots = []
    j0 = 0
    for k in kinds:
        slots.append((k, j0))
        j0 += 2 if k == "P" else 4
    assert j0 == 128
    return slots


SLOTS = _slot_plan()

_CACHE = {}


def _build_module():
    nc = bacc.Bacc("TRN2", target_bir_lowering=False, debug=False, num_devices=NCORES)

    # ---- per-core data ----
    nodes_t = nc.dram_tensor("nodes_t", [D, N], F32, kind="ExternalInput")
    e30 = nc.dram_tensor("e30", [N, N], BF16, kind="ExternalInput")
    # ---- parameters / host-prepared constants (same on all cores) ----
    # wpack1 [128, 128] f32: cols 0:64 = w1_nb, 64:128 = w1_self
    wpack1 = nc.dram_tensor("wpack1", [128, 128], F32, kind="ExternalInput")
    # wpack2 [64, 258] f32: w2_nb | w2_self | W_vc | W_uc | cV | cU
    # (W_vc = nb_w2 @ comb_w1[H:], W_uc = self_w2 @ comb_w1[:H] -- the two
    # linear layers collapsed; cV = comb_w1[H:].T @ nb_b2,
    # cU = comb_w1[:H].T @ self_b2 + comb_b1)
    wpack2 = nc.dram_tensor("wpack2", [H, 258], F32, kind="ExternalInput")
    # bfpack [64, 776] bf16: col 0 = 0.2*w2c; row 0: ones(512) @ 8,
    # self_b2 @ 520, nb_b2 @ 584, nb_b1 @ 648, self_b1 @ 712
    bfpack = nc.dram_tensor("bfpack", [H, 776], BF16, kind="ExternalInput")
    w2bd_dr = nc.dram_tensor("w2bd_dr", [128, 2, 252], FP8, kind="ExternalInput")
    w2bd_bf = nc.dram_tensor("w2bd_bf", [128, 254], BF16, kind="ExternalInput")
    id_bf16 = nc.dram_tensor("id_bf16", [128, 128], BF16, kind="ExternalInput")

    out = nc.dram_tensor("out", [N, H], F32, kind="ExternalOutput")

    with tile.TileContext(nc) as tc:
        _emit(nc, tc, locals())
    nc.compile()
    return nc


def _emit(nc, tc, t):
    AF = mybir.ActivationFunctionType
    OP = mybir.AluOpType

    def R(ap):
        return ap.bitcast(F32R)

    with (
        tc.tile_pool(name="persist", bufs=1) as P,
        tc.tile_pool(name="ework", bufs=2) as EW,
        tc.tile_pool(name="relu", bufs=12) as RL,
        tc.tile_pool(name="small", bufs=4) as SM,
        tc.tile_pool(name="psumR", bufs=2, space="PSUM") as PR,
        tc.tile_pool(name="psumT", bufs=2, space="PSUM") as PT,
        tc.tile_pool(name="psumA", bufs=1, space="PSUM") as PA,
    ):
        # ---------- load constants (DMA issue costs 500ns of engine time;
        # spread queues and order by need-time) ----------
        def load(name, shape, dtype, eng):
            tl = P.tile(shape, dtype, tag=name, name=name + "_t")
            eng.dma_start(out=tl[:], in_=t[name].ap())
            return tl

        xT = P.tile([D, N], F32, tag="nodes_t", name="xT")
        nc.sync.dma_start(out=xT[:, 0:256], in_=t["nodes_t"].ap()[:, 0:256])
        nc.scalar.dma_start(out=xT[:, 256:512], in_=t["nodes_t"].ap()[:, 256:512])
        wp1 = load("wpack1", [128, 128], F32, nc.scalar)
        w1n, w1s = wp1[:, 0:H], wp1[:, H:128]
        wp2 = load("wpack2", [H, 258], F32, nc.sync)
        w2n, w2s = wp2[:, 0:H], wp2[:, H:2 * H]
        wvc, wuc = wp2[:, 2 * H:3 * H], wp2[:, 3 * H:4 * H]
        cV, cU = wp2[:, 256:257], wp2[:, 257:258]
        bfp = load("bfpack", [H, 776], BF16, nc.sync)
        w2c02 = bfp[:, 0:1]
        ones512 = bfp[0:1, 8:520]
        onesr = bfp[0:1, 8:136]
        b2sr, b2nr = bfp[0:1, 520:584], bfp[0:1, 584:648]
        b1nr, b1sr = bfp[0:1, 648:712], bfp[0:1, 712:776]
        wbf = load("w2bd_bf", [128, 254], BF16, nc.sync)
        e30all = P.tile([128, NT, N], BF16, tag="e30all", name="e30all")
        nc.sync.dma_start(
            out=e30all[:],
            in_=t["e30"].ap().rearrange("(t p) i -> p t i", t=NT))
        e30t = [e30all[:, jt, :] for jt in range(NT)]
        idb = load("id_bf16", [128, 128], BF16, nc.gpsimd)
        wdr = load("w2bd_dr", [128, 2, 252], FP8, nc.gpsimd)

        # preload the ACT function table during the idle DMA window so the
        # one-time LoadActFuncSet doesn't land on the critical path
        scrap = SM.tile([1, 2], F32, tag="scrap", name="scrap")
        nc.vector.memset(scrap[:], 0.0)
        scrap2 = SM.tile([1, 2], BF16, tag="scrap2", name="scrap2")
        nc.scalar.activation(out=scrap2[:], in_=scrap[:], func=AF.Exp)

        # PE p-state warm-up: the tensor engine only reaches 2.4GHz after
        # 3us of continuous execution, and phase 1 would otherwise run its
        # matmuls at 0.65-1.2GHz. Tread water on scrap data until the first
        # real matmul's inputs arrive.
        n_wu = int(os.environ.get("GAT_WARMUP", "84"))
        if n_wu:
            wu_w = SM.tile([H, H], BF16, tag="wu_w", name="wu_w")
            nc.vector.memset(wu_w[:], 0.0)
            wu_ps = PT.tile([H, H], F32, tag="pt", name="wu_ps",
                            padded_shape=[128, 128])
            for _ in range(n_wu):
                nc.tensor.matmul(wu_ps[:], wu_w[:], wu_w[:], start=True, stop=True)

        # ---------- MLPs (transposed; h on partitions; float32r matmuls;
        # b1 biases folded into PSUM via K=1 ones-matmuls so leaky is one
        # Pool op straight from PSUM; layers 2+3 collapsed on the host) ----
        pm_s1 = PR.tile([H, N], F32, tag="psumR", name="pm_s1",
                        padded_shape=[128, N])
        nc.tensor.matmul(pm_s1[:], R(w1s[:]), R(xT[:]), start=True, stop=False)
        nc.tensor.matmul(pm_s1[:], b1sr, ones512, start=False, stop=True)
        pm_n1 = PR.tile([H, N], F32, tag="psumR", name="pm_n1",
                        padded_shape=[128, N])
        nc.tensor.matmul(pm_n1[:], R(w1n[:]), R(xT[:]), start=True, stop=False)
        nc.tensor.matmul(pm_n1[:], b1nr, ones512, start=False, stop=True)
        h1T_s = P.tile([H, N], F32, tag="h1T_s")
        nc.gpsimd.scalar_tensor_tensor(out=h1T_s[:], in0=pm_s1[:], scalar=0.2,
                                       in1=pm_s1[:], op0=OP.mult, op1=OP.max)
        h1T_n = P.tile([H, N], F32, tag="h1T_n")
        nc.vector.scalar_tensor_tensor(out=h1T_n[:], in0=pm_n1[:], scalar=0.2,
                                       in1=pm_n1[:], op0=OP.mult, op1=OP.max)

        pm_u = PR.tile([H, N], F32, tag="psumR", name="pm_u",
                       padded_shape=[128, N])
        nc.tensor.matmul(pm_u[:], R(wuc[:]), R(h1T_s[:]), start=True, stop=True)
        Urep = P.tile([128, N], BF16, tag="Urep")
        nc.scalar.activation(out=Urep[:H, :], in_=pm_u[:H, :], func=AF.Identity,
                             bias=cU, scale=1.0)
        nc.gpsimd.tensor_scalar_add(out=Urep[H:, :], in0=pm_u[:H, :], scalar1=cU)

        pm_v = PR.tile([H, N], F32, tag="psumR", name="pm_v",
                       padded_shape=[128, N])
        nc.tensor.matmul(pm_v[:], R(wvc[:]), R(h1T_n[:]), start=True, stop=True)
        V2 = P.tile([128, N // 2], F32, tag="V2")
        vsplit = pm_v[:H, :].rearrange("p (jp g) -> p jp g", g=2)
        nc.vector.tensor_scalar_add(out=V2[:H, :], in0=vsplit[:, :, 0], scalar1=cV)
        nc.vector.tensor_scalar_add(out=V2[H:, :], in0=vsplit[:, :, 1], scalar1=cV)
        V_T = P.tile([H, N], BF16, tag="V_T")
        nc.gpsimd.tensor_scalar_add(out=V_T[:], in0=pm_v[:], scalar1=cV)

        # svb[j, jt] = 0.2 * w2 . V_j  (exp bias, per-partition j)
        psv = PT.tile([128, NT], F32, tag="pt", name="psv", padded_shape=[128, 128])
        for jt in range(NT):
            nc.tensor.matmul(psv[:, jt:jt + 1], V_T[:, bass.ts(jt, 128)],
                             w2c02[:], start=True, stop=True)

        # self_e [row, H] f32 and nb_e65 [row, H+1] bf16 (ones col -> denom)
        selfe, nbe65, pa_se, pa_ne = [], [], [], []
        for it in range(NT):
            pa = PT.tile([128, H], F32, tag="pt", name=f"pa_s{it}",
                         padded_shape=[128, 128])
            nc.tensor.matmul(pa[:], R(h1T_s[:, bass.ts(it, 128)]), R(w2s[:]),
                             start=True, stop=False)
            nc.tensor.matmul(pa[:], onesr, b2sr, start=False, stop=True)
            pa_se.append(pa)
            se = P.tile([128, H], F32, tag=f"selfe{it}", name=f"selfe{it}")
            selfe.append(se)
        for jt in range(NT):
            pa = PT.tile([128, H], F32, tag="pt", name=f"pa_n{jt}",
                         padded_shape=[128, 128])
            nc.tensor.matmul(pa[:], R(h1T_n[:, bass.ts(jt, 128)]), R(w2n[:]),
                             start=True, stop=False)
            nc.tensor.matmul(pa[:], onesr, b2nr, start=False, stop=True)
            pa_ne.append(pa)
            ne = P.tile([128, H + 1], BF16, tag=f"nbe{jt}", name=f"nbe{jt}")
            nc.gpsimd.memset(ne[:, H:H + 1], 1.0)
            nbe65.append(ne)

        svb = SM.tile([128, NT], F32, tag="svb")

        # aux copy work, deferred into the jt=0 build stream so in-order
        # engine queues never block on not-yet-ready PSUM producers
        aux = {
            4: lambda: nc.vector.tensor_copy(out=svb[:], in_=psv[:, 0:NT]),
            6: lambda: nc.gpsimd.tensor_scalar_add(out=selfe[0][:],
                                                   in0=pa_se[0][:], scalar1=0.0),
            8: lambda: nc.gpsimd.tensor_scalar_add(out=selfe[1][:],
                                                   in0=pa_se[1][:], scalar1=0.0),
            10: lambda: nc.gpsimd.tensor_scalar_add(out=selfe[2][:],
                                                    in0=pa_se[2][:], scalar1=0.0),
            12: lambda: nc.gpsimd.tensor_scalar_add(out=selfe[3][:],
                                                    in0=pa_se[3][:], scalar1=0.0),
            14: lambda: nc.vector.tensor_copy(out=nbe65[0][:, 0:H], in_=pa_ne[0][:]),
            18: lambda: nc.vector.tensor_copy(out=nbe65[1][:, 0:H], in_=pa_ne[1][:]),
            22: lambda: nc.vector.tensor_copy(out=nbe65[2][:, 0:H], in_=pa_ne[2][:]),
            26: lambda: nc.vector.tensor_copy(out=nbe65[3][:, 0:H], in_=pa_ne[3][:]),
        }

        # ---------- main pass: j-major scores -> exp -> aggregation ----------
        X = [P.tile([128, N], BF16, tag=f"X{jt}", name=f"X{jt}") for jt in range(NT)]
        pagg = [PA.tile([128, H + 1], F32, tag=f"pagg{it}", name=f"pagg{it}",
                        padded_shape=[128, N]) for it in range(NT)]

        def agg(jt):
            for it in range(NT):
                nc.tensor.matmul(pagg[it][:], X[jt][:, bass.ts(it, 128)],
                                 nbe65[jt][:], start=(jt == 0), stop=False)

        n_mm = len(SLOTS)
        # mask/diag bias injects for the first two tiles run during the
        # phase-1 PE idle window
        ps_list = {}
        for jt in range(2):
            ps_list[jt] = PR.tile([128, N], F32, tag="psumR", name=f"ps{jt}")
            nc.tensor.matmul(ps_list[jt][:], idb[:], e30t[jt], start=True,
                             stop=False)
        n_wu2 = int(os.environ.get("GAT_WARMUP2", "72"))
        if n_wu2:
            wu_w2 = SM.tile([H, H], BF16, tag="wu_w2", name="wu_w2")
            nc.vector.memset(wu_w2[:], 0.0)
            wu_ps2 = PT.tile([H, H], F32, tag="pt", name="wu_ps2",
                             padded_shape=[128, 128])
            for _ in range(n_wu2):
                nc.tensor.matmul(wu_ps2[:], wu_w2[:], wu_w2[:], start=True,
                                 stop=True)
        qi = pi = 0
        for jt in range(NT):
            if jt in ps_list:
                ps = ps_list[jt]
            else:
                ps = PR.tile([128, N], F32, tag="psumR", name="ps")
                nc.tensor.matmul(ps[:], idb[:], e30t[jt], start=True, stop=False)
            for k, (kind, j0) in enumerate(SLOTS):
                if jt == 0 and k in aux:
                    aux[k]()
                if jt > 0 and k == 3:
                    # delayed one tile so ACT/PE never stall at the boundary
                    nc.scalar.activation(out=X[jt - 1][:], in_=ps_prev[:],
                                         func=AF.Exp,
                                         bias=svb[:, jt - 1:jt], scale=1.0)
                if jt > 0 and k == 8:
                    agg(jt - 1)
                last = k == n_mm - 1
                if kind == "P":
                    rl = RL.tile([128, N], BF16, tag="rl", name="rl")
                    eng = PAIR_PAT[pi % len(PAIR_PAT)]
                    pi += 1
                    c = jt * 64 + j0 // 2
                    _build(nc, eng, rl[:], Urep[:], V2[:, c:c + 1], AF, OP)
                    nc.tensor.matmul(ps[:], wbf[:, 126 - j0:254 - j0], rl[:],
                                     start=False, stop=last)
                else:
                    rl4 = RL.tile([128, 2, N], FP8, tag="rl4", name="rl4")
                    for q in range(2):
                        eng = QUAD_PAT[qi % len(QUAD_PAT)]
                        qi += 1
                        c = jt * 64 + j0 // 2 + q
                        _build(nc, eng, rl4[:, q, :], Urep[:],
                               V2[:, c:c + 1], AF, OP)
                    nc.tensor.matmul(ps[:], wdr[:, :, 124 - j0:252 - j0], rl4[:],
                                     perf_mode=mybir.MatmulPerfMode.DoubleRow,
                                     start=False, stop=last)
            ps_prev = ps

        # ---------- tail: chunked last exp -> per-chunk agg -> assembly ----
        out_engs = [nc.sync, nc.scalar, nc.gpsimd, nc.sync]
        for it in range(NT):
            nc.scalar.activation(out=X[NT - 1][:, bass.ts(it, 128)],
                                 in_=ps_prev[:, bass.ts(it, 128)], func=AF.Exp,
                                 bias=svb[:, NT - 1:NT], scale=1.0)
            nc.tensor.matmul(pagg[it][:], X[NT - 1][:, bass.ts(it, 128)],
                             nbe65[NT - 1][:], start=False, stop=True)
            den = pagg[it][:, H:H + 1]
            gate = SM.tile([128, 1], F32, tag="gate", name="gate")
            nc.gpsimd.tensor_single_scalar(out=gate[:], in_=den, scalar=1e-6,
                                           op=OP.is_gt)
            dsafe = SM.tile([128, 1], F32, tag="dsafe", name="dsafe")
            nc.gpsimd.tensor_scalar_max(out=dsafe[:], in0=den, scalar1=1e-6)
            recip = SM.tile([128, 1], F32, tag="recip", name="recip")
            nc.vector.reciprocal(out=recip[:], in_=dsafe[:])
            sg = SM.tile([128, H], F32, tag="sg")
            nc.gpsimd.tensor_scalar_mul(out=sg[:], in0=selfe[it][:], scalar1=gate[:])
            ot = SM.tile([128, H], F32, tag="ot")
            nc.gpsimd.scalar_tensor_tensor(out=ot[:], in0=pagg[it][:, 0:H],
                                           scalar=recip[:], in1=sg[:],
                                           op0=OP.mult, op1=OP.add)
            out_engs[it].dma_start(out=t["out"].ap()[bass.ts(it, 128), :], in_=ot[:])


def _build(nc, eng, out_ap, urep_ap, v2col, AF, OP):
    """relu(Urep + V2[:, col]) on the chosen engine."""
    if eng == "v":
        nc.vector.tensor_scalar(out=out_ap, in0=urep_ap, scalar1=v2col,
                                scalar2=0.0, op0=OP.add, op1=OP.max)
    elif eng == "a":
        nc.scalar.activation(out=out_ap, in_=urep_ap, func=AF.Relu,
                             bias=v2col, scale=1.0)
    else:
        nc.gpsimd.tensor_scalar(out=out_ap, in0=urep_ap, scalar1=v2col,
                                scalar2=0.0, op0=OP.add, op1=OP.max)


def _host_constants(inputs):
    f32 = np.float32
    bf = ml_dtypes.bfloat16
    H_ = H
    w2 = np.asarray(inputs["comb_w2"], f32)            # [H, 1]
    w08 = 0.8 * w2[:, 0]
    wdr = np.zeros((128, 2, 252), f32)
    for g in range(2):
        for q in range(2):
            wdr[g * H_:(g + 1) * H_, q, 124 + 2 * q + g] = w08
    wbf = np.zeros((128, 254), f32)
    wbf[0:H_, 126] = w08
    wbf[H_:128, 127] = w08
    wpack1 = np.concatenate([np.asarray(inputs["nb_w1"], f32),
                             np.asarray(inputs["self_w1"], f32)], axis=1)
    bvec = np.stack([
        np.asarray(inputs["self_b1"], f32),
        np.asarray(inputs["nb_b1"], f32),
        np.asarray(inputs["self_b2"], f32),
        np.asarray(inputs["nb_b2"], f32),
        np.asarray(inputs["comb_b1"], f32),
    ], axis=1)
    w1c_s = np.asarray(inputs["comb_w1"], f32)[:H_]     # [H, H]
    w1c_n = np.asarray(inputs["comb_w1"], f32)[H_:]
    w2s_ = np.asarray(inputs["self_w2"], f32)
    w2n_ = np.asarray(inputs["nb_w2"], f32)
    cV = w1c_n.T @ np.asarray(inputs["nb_b2"], f32)
    cU = w1c_s.T @ np.asarray(inputs["self_b2"], f32) + np.asarray(
        inputs["comb_b1"], f32)
    wpack2 = np.concatenate([
        w2n_, w2s_, w2n_ @ w1c_n, w2s_ @ w1c_s,
        cV[:, None], cU[:, None],
    ], axis=1)
    bfpack = np.zeros((H_, 776), f32)
    bfpack[:, 0] = 0.2 * w2[:, 0]
    bfpack[0, 8:520] = 1.0
    bfpack[0, 520:584] = np.asarray(inputs["self_b2"], f32)
    bfpack[0, 584:648] = np.asarray(inputs["nb_b2"], f32)
    bfpack[0, 648:712] = np.asarray(inputs["nb_b1"], f32)
    bfpack[0, 712:776] = np.asarray(inputs["self_b1"], f32)
    consts = {
        "wpack1": np.ascontiguousarray(wpack1),
        "wpack2": np.ascontiguousarray(wpack2),
        "bfpack": bfpack.astype(bf),
        "w2bd_dr": wdr.astype(NP_FP8),
        "w2bd_bf": wbf.astype(bf),
        "id_bf16": np.eye(128, dtype=f32).astype(bf),
    }
    return consts


def _host_percore(inputs):
    """Per-core tensors: transposed nodes + additive mask bias."""
    f32 = np.float32
    bf = ml_dtypes.bfloat16
    nodes = np.asarray(inputs["nodes"], f32).reshape(B, N, D)
    nodes_t = np.ascontiguousarray(nodes.transpose(0, 2, 1))      # [B, D, N]
    mask = (np.asarray(inputs["edges"]) != 0)
    mask &= ~np.eye(N, dtype=bool)[None]
    e30 = np.where(mask, np.float32(0.0), np.float32(-30.0)).astype(bf)
    return nodes_t, e30


def _build_fast_path(nc):
    """Cache a single jitted shard_map executable so repeat kernel() calls
    skip jax re-tracing (same lowering run_bass_kernel_spmd uses under axon)."""
    import jax
    from jax.sharding import Mesh, PartitionSpec
    from jax.experimental.shard_map import shard_map

    bass2jax.install_neuronx_cc_hook()
    pname = nc.partition_id_tensor.name if nc.partition_id_tensor else None
    in_names, out_names, out_avals = [], [], []
    for alloc in nc.m.functions[0].allocations:
        if not isinstance(alloc, mybir.MemoryLocationSet):
            continue
        name = alloc.memorylocations[0].name
        if alloc.kind == "ExternalInput":
            if name != pname:
                in_names.append(name)
        elif alloc.kind == "ExternalOutput":
            out_names.append(name)
            out_avals.append(jax.core.ShapedArray(tuple(alloc.tensor_shape),
                                                  mybir.dt.np(alloc.dtype)))
    all_names = in_names + out_names + ([pname] if pname else [])

    def _body(*args):
        operands = list(args)
        if pname is not None:
            operands.append(bass2jax.partition_id_tensor())
        return tuple(bass2jax._bass_exec_p.bind(
            *operands, out_avals=tuple(out_avals), in_names=tuple(all_names),
            out_names=tuple(out_names), lowering_input_output_aliases=(),
            sim_require_finite=True, sim_require_nnan=True, nc=nc))

    devices = jax.devices()[:NCORES]
    mesh = Mesh(np.asarray(devices), ("core",))
    n_io = len(in_names) + len(out_names)
    sharded = jax.jit(
        shard_map(_body, mesh=mesh, in_specs=(PartitionSpec("core"),) * n_io,
                  out_specs=(PartitionSpec("core"),) * len(out_names),
                  check_rep=False),
        keep_unused=True,
    )
    return sharded, in_names, out_names, out_avals


def kernel(**inputs):
    first = "nc" not in _CACHE
    if first:
        _CACHE["nc"] = _build_module()
    nc = _CACHE["nc"]

    consts = _host_constants(inputs)
    nodes_t, e30 = _host_percore(inputs)

    in_maps = []
    for c in range(NCORES):
        m = dict(consts)
        m["nodes_t"] = nodes_t[c]
        m["e30"] = e30[c]
        in_maps.append(m)

    if first:
        res = run_bass_kernel_spmd(nc, in_maps, core_ids=list(range(NCORES)))
        _CACHE["fast"] = _build_fast_path(nc)
        return np.stack([res.results[c]["out"] for c in range(NCORES)]).astype(np.float32)

    import jax
    sharded, in_names, out_names, out_avals = _CACHE["fast"]
    ckey = hash(tuple((k, v.tobytes()) for k, v in sorted(consts.items())))
    if _CACHE.get("ckey") != ckey:
        _CACHE["cdev"] = {
            n: jax.device_put(np.concatenate([np.asarray(in_maps[c][n])
                                              for c in range(NCORES)], axis=0))
            for n in in_names if n not in ("nodes_t", "e30")
        }
        _CACHE["zdev"] = [jax.device_put(np.zeros((NCORES * a.shape[0], *a.shape[1:]),
                                                  a.dtype)) for a in out_avals]
        _CACHE["ckey"] = ckey
    cdev = _CACHE["cdev"]
    concat_in = [cdev[n] if n in cdev else
                 np.concatenate([np.asarray(in_maps[c][n]) for c in range(NCORES)], axis=0)
                 for n in in_names]
    outs = sharded(*concat_in, *_CACHE["zdev"])
    i = out_names.index("out")
    return np.asarray(outs[i]).reshape(NCORES, N, H).astype(np.float32)


# revision 16
# speedup vs baseline: 1.0529x; 1.0529x over previous
"""GAT message-passing kernel for Trainium2 (8 NeuronCores, data-parallel over batch).

Math (per batch element b, derived from the reference nn.Module):
    x      = nodes.reshape(N, D)
    self_e = mlp2(x, self_*)                 # [N, H]
    nb_e   = mlp2(x, nb_*)                   # [N, H]
    U      = self_e @ comb_w1[:H] + b1c      # [N, H]  (i side)
    V      = nb_e @ comb_w1[H:]              # [N, H]  (j side)
    scores(i,j) = leaky(U_i + V_j) @ w2 + const_i
                = 0.8*relu(U_i+V_j)@w2 + 0.2*w2.V_j + const_i
    Softmax over j drops per-i constants, so
      X^T[j,i] = exp(0.8 relu(U_i+V_j)@w2 + 0.2 w2.V_j - 30*(1-mask[j,i]))
    where mask[j,i] = edges[j,i]*(j!=i); the -30 bias (host-precomputed
    [N,N] bf16 tile, injected into PSUM by one identity matmul per j-tile)
    replaces all explicit mask multiplies: masked entries come out ~e-13.
    denom[i] = sum_j X^T[j,i] lands in the aggregation matmul via a ones
    column appended to nb_e. (|scores| < 2.5, so exp needs no max-sub.)
    out[i] = gate*(recip * X^T(:,i).nb_e + self_e), gate = denom > 1e-6.

Device mapping (one core per batch element, j-major pairwise stage):
  - Transposed (g,h)-on-partitions layout: partitions = (j-parity g, h), so one
    tensor_scalar(add,max) builds relu(U + V_j) for TWO j's at once as a
    [128, 512] tile over all i. Scores come out j-on-partitions, which is the
    exact layout exp/mask/aggregation need - no PE transposes anywhere.
  - PE reduces over (g,h) two ways, mixed freely inside one 128-row PSUM
    accumulation group per j-tile:
      * bf16 "pair" slots: [128,512] bf16 builds (DVE 4x mode, 194ns) with a
        2-column sliding-window block-diagonal 0.8*w2 lhsT (512cy/matmul);
      * fp8e4 DoubleRow "quad" slots: [128,2,512] fp8 builds (two j-pairs) with
        a [128,2,128] sliding-window lhsT, K=256 contraction at 0.5 cyc/row
        (256cy/matmul = 4 j's) - 4x the bf16 PE throughput.
    Slot kinds interleave (P,Q,P,Q,...) so PE consumption matches the
    aggregate build rate of DVE/ACT/Pool (pattern env-tunable).
  - exp runs on ACT straight out of PSUM with bias = 0.2*w2.V_j (per-partition,
    computed by 4 tiny K=64 matmuls), output bf16 X^T[j,i] used directly as
    the stationary operand of the aggregation matmuls.
  - Aggregation rhs nb_e carries an appended ones column: out[i,64] = denom_i
    lands per-partition in PSUM; gate/recip/assembly are a few [128,64] DVE ops.
  - MLP/U/V precompute runs float32r (full fp32 data, 1 cyc/row at free>=256);
    self_e stays fp32 end-to-end (it adds directly into the output).
"""

import os
import sys

sys.path.insert(0, "/opt/trn_rl_repo")

import numpy as np
import ml_dtypes

import concourse.bass as bass
import concourse.bacc as bacc
import concourse.tile as tile
from concourse import mybir, bass2jax
from concourse.bass_utils import run_bass_kernel_spmd

B, N, H, D = 8, 512, 64, 128
NCORES = 8
NT = N // 128          # 4 j/i tiles of 128
F32 = mybir.dt.float32
F32R = mybir.dt.float32r
BF16 = mybir.dt.bfloat16
FP8 = mybir.dt.float8e4
NP_FP8 = mybir.dt.np(FP8)

# Per-j-tile slot plan: P bf16 pairs (2 rows each) + Q fp8 DoubleRow quads
# (4 rows each), 2P + 4Q = 128.
NPAIRS = int(os.environ.get("GAT_P", "26"))
NQUADS = (128 - 2 * NPAIRS) // 4
assert 2 * NPAIRS + 4 * NQUADS == 128

# Build-engine schedule: 'v' = VectorE, 'a' = ScalarE, 'p' = gpsimd/Pool.
PAIR_PAT = os.environ.get("GAT_PAIR_PAT", "v" * NPAIRS)
QUAD_PAT = os.environ.get("GAT_QUAD_PAT", "")


def _smooth_seq(counts):
    """Smooth weighted round-robin sequence over engine keys."""
    total = sum(counts.values())
    acc = {k: 0.0 for k in counts}
    seq = []
    rem = dict(counts)
    for _ in range(total):
        for k in counts:
            acc[k] += counts[k] / total
        k = max((kk for kk in counts if rem[kk] > 0), key=lambda kk: acc[kk])
        acc[k] -= 1.0
        rem[k] -= 1
        seq.append(k)
    return "".join(seq)


if not QUAD_PAT:
    nq2 = 2 * NQUADS * NT          # consumed globally across all j-tiles
    np_ = (nq2 * 72 + 76) // 152
    na = (nq2 * 47 + 76) // 152
    nv = max(nq2 - np_ - na, 0)
    QUAD_PAT = _smooth_seq({"p": np_, "a": na, "v": nv} if nv else {"p": np_, "a": na})


def _slot_plan():
    """Interleaved (kind, local_j0) slots covering 128 rows."""
    kinds = _smooth_seq({"P": NPAIRS, "Q": NQUADS})
    slots = []
    j0 = 0
    for k in kinds:
        slots.append((k, j0))
        j0 += 2 if k == "P" else 4
    assert j0 == 128
    return slots


SLOTS = _slot_plan()

_CACHE = {}


def _build_module():
    nc = bacc.Bacc("TRN2", target_bir_lowering=False, debug=False, num_devices=NCORES)

    # ---- per-core data ----
    nodes_t = nc.dram_tensor("nodes_t", [D, N], F32, kind="ExternalInput")
    e30 = nc.dram_tensor("e30", [N, N], BF16, kind="ExternalInput")
    # ---- parameters / host-prepared constants (same on all cores) ----
    # wpack1 [128, 128] f32: cols 0:64 = w1_nb, 64:128 = w1_self
    wpack1 = nc.dram_tensor("wpack1", [128, 128], F32, kind="ExternalInput")
    # wpack2 [64, 258] f32: w2_nb | w2_self | W_vc | W_uc | cV | cU
    # (W_vc = nb_w2 @ comb_w1[H:], W_uc = self_w2 @ comb_w1[:H] -- the two
    # linear layers collapsed; cV = comb_w1[H:].T @ nb_b2,
    # cU = comb_w1[:H].T @ self_b2 + comb_b1)
    wpack2 = nc.dram_tensor("wpack2", [H, 258], F32, kind="ExternalInput")
    # bfpack [64, 776] bf16: col 0 = 0.2*w2c; row 0: ones(512) @ 8,
    # self_b2 @ 520, nb_b2 @ 584, nb_b1 @ 648, self_b1 @ 712
    bfpack = nc.dram_tensor("bfpack", [H, 776], BF16, kind="ExternalInput")
    w2bd_dr = nc.dram_tensor("w2bd_dr", [128, 2, 252], FP8, kind="ExternalInput")
    w2bd_bf = nc.dram_tensor("w2bd_bf", [128, 254], BF16, kind="ExternalInput")
    id_bf16 = nc.dram_tensor("id_bf16", [128, 128], BF16, kind="ExternalInput")

    out = nc.dram_tensor("out", [N, H], F32, kind="ExternalOutput")

    with tile.TileContext(nc) as tc:
        _emit(nc, tc, locals())
    nc.compile()
    return nc


def _emit(nc, tc, t):
    AF = mybir.ActivationFunctionType
    OP = mybir.AluOpType

    def R(ap):
        return ap.bitcast(F32R)

    with (
        tc.tile_pool(name="persist", bufs=1) as P,
        tc.tile_pool(name="ework", bufs=2) as EW,
        tc.tile_pool(name="relu", bufs=12) as RL,
        tc.tile_pool(name="small", bufs=4) as SM,
        tc.tile_pool(name="psumR", bufs=2, space="PSUM") as PR,
        tc.tile_pool(name="psumT", bufs=2, space="PSUM") as PT,
        tc.tile_pool(name="psumA", bufs=1, space="PSUM") as PA,
    ):
        # ---------- load constants (DMA issue costs 500ns of engine time;
        # spread queues and order by need-time) ----------
        def load(name, shape, dtype, eng):
            tl = P.tile(shape, dtype, tag=name, name=name + "_t")
            eng.dma_start(out=tl[:], in_=t[name].ap())
            return tl

        xT = P.tile([D, N], F32, tag="nodes_t", name="xT")
        nc.sync.dma_start(out=xT[:, 0:256], in_=t["nodes_t"].ap()[:, 0:256])
        nc.scalar.dma_start(out=xT[:, 256:512], in_=t["nodes_t"].ap()[:, 256:512])
        wp1 = load("wpack1", [128, 128], F32, nc.scalar)
        w1n, w1s = wp1[:, 0:H], wp1[:, H:128]
        wp2 = load("wpack2", [H, 258], F32, nc.sync)
        w2n, w2s = wp2[:, 0:H], wp2[:, H:2 * H]
        wvc, wuc = wp2[:, 2 * H:3 * H], wp2[:, 3 * H:4 * H]
        cV, cU = wp2[:, 256:257], wp2[:, 257:258]
        bfp = load("bfpack", [H, 776], BF16, nc.sync)
        w2c02 = bfp[:, 0:1]
        ones512 = bfp[0:1, 8:520]
        onesr = bfp[0:1, 8:136]
        b2sr, b2nr = bfp[0:1, 520:584], bfp[0:1, 584:648]
        b1nr, b1sr = bfp[0:1, 648:712], bfp[0:1, 712:776]
        wbf = load("w2bd_bf", [128, 254], BF16, nc.sync)
        e30all = P.tile([128, NT, N], BF16, tag="e30all", name="e30all")
        nc.sync.dma_start(
            out=e30all[:],
            in_=t["e30"].ap().rearrange("(t p) i -> p t i", t=NT))
        e30t = [e30all[:, jt, :] for jt in range(NT)]
        idb = load("id_bf16", [128, 128], BF16, nc.gpsimd)
        wdr = load("w2bd_dr", [128, 2, 252], FP8, nc.gpsimd)

        # preload the ACT function table during the idle DMA window so the
        # one-time LoadActFuncSet doesn't land on the critical path
        scrap = SM.tile([1, 2], F32, tag="scrap", name="scrap")
        nc.vector.memset(scrap[:], 0.0)
        scrap2 = SM.tile([1, 2], BF16, tag="scrap2", name="scrap2")
        nc.scalar.activation(out=scrap2[:], in_=scrap[:], func=AF.Exp)

        # PE p-state warm-up: the tensor engine only reaches 2.4GHz after
        # 3us of continuous execution, and phase 1 would otherwise run its
        # matmuls at 0.65-1.2GHz. Tread water on scrap data until the first
        # real matmul's inputs arrive.
        n_wu = int(os.environ.get("GAT_WARMUP", "84"))
        if n_wu:
            wu_w = SM.tile([H, H], BF16, tag="wu_w", name="wu_w")
            nc.vector.memset(wu_w[:], 0.0)
            wu_ps = PT.tile([H, H], F32, tag="pt", name="wu_ps",
                            padded_shape=[128, 128])
            for _ in range(n_wu):
                nc.tensor.matmul(wu_ps[:], wu_w[:], wu_w[:], start=True, stop=True)

        # ---------- MLPs (transposed; h on partitions; float32r matmuls;
        # b1 biases folded into PSUM via K=1 ones-matmuls so leaky is one
        # Pool op straight from PSUM; layers 2+3 collapsed on the host) ----
        pm_s1 = PR.tile([H, N], F32, tag="psumR", name="pm_s1",
                        padded_shape=[128, N])
        nc.tensor.matmul(pm_s1[:], R(w1s[:]), R(xT[:]), start=True, stop=False)
        nc.tensor.matmul(pm_s1[:], b1sr, ones512, start=False, stop=True)
        pm_n1 = PR.tile([H, N], F32, tag="psumR", name="pm_n1",
                        padded_shape=[128, N])
        nc.tensor.matmul(pm_n1[:], R(w1n[:]), R(xT[:]), start=True, stop=False)
        nc.tensor.matmul(pm_n1[:], b1nr, ones512, start=False, stop=True)
        h1T_s = P.tile([H, N], F32, tag="h1T_s")
        nc.gpsimd.scalar_tensor_tensor(out=h1T_s[:], in0=pm_s1[:], scalar=0.2,
                                       in1=pm_s1[:], op0=OP.mult, op1=OP.max)
        h1T_n = P.tile([H, N], F32, tag="h1T_n")
        nc.vector.scalar_tensor_tensor(out=h1T_n[:], in0=pm_n1[:], scalar=0.2,
                                       in1=pm_n1[:], op0=OP.mult, op1=OP.max)

        pm_u = PR.tile([H, N], F32, tag="psumR", name="pm_u",
                       padded_shape=[128, N])
        nc.tensor.matmul(pm_u[:], R(wuc[:]), R(h1T_s[:]), start=True, stop=True)
        Urep = P.tile([128, N], BF16, tag="Urep")
        nc.scalar.activation(out=Urep[:H, :], in_=pm_u[:H, :], func=AF.Identity,
                             bias=cU, scale=1.0)
        nc.gpsimd.tensor_scalar_add(out=Urep[H:, :], in0=pm_u[:H, :], scalar1=cU)

        pm_v = PR.tile([H, N], F32, tag="psumR", name="pm_v",
                       padded_shape=[128, N])
        nc.tensor.matmul(pm_v[:], R(wvc[:]), R(h1T_n[:]), start=True, stop=True)
        V2 = P.tile([128, N // 2], F32, tag="V2")
        vsplit = pm_v[:H, :].rearrange("p (jp g) -> p jp g", g=2)
        nc.vector.tensor_scalar_add(out=V2[:H, :], in0=vsplit[:, :, 0], scalar1=cV)
        nc.vector.tensor_scalar_add(out=V2[H:, :], in0=vsplit[:, :, 1], scalar1=cV)
        V_T = P.tile([H, N], BF16, tag="V_T")
        nc.gpsimd.tensor_scalar_add(out=V_T[:], in0=pm_v[:], scalar1=cV)

        # svb[j, jt] = 0.2 * w2 . V_j  (exp bias, per-partition j)
        psv = PT.tile([128, NT], F32, tag="pt", name="psv", padded_shape=[128, 128])
        for jt in range(NT):
            nc.tensor.matmul(psv[:, jt:jt + 1], V_T[:, bass.ts(jt, 128)],
                             w2c02[:], start=True, stop=True)

        # self_e [row, H] f32 and nb_e65 [row, H+1] bf16 (ones col -> denom)
        selfe, nbe65, pa_se, pa_ne = [], [], [], []
        for it in range(NT):
            pa = PT.tile([128, H], F32, tag="pt", name=f"pa_s{it}",
                         padded_shape=[128, 128])
            nc.tensor.matmul(pa[:], R(h1T_s[:, bass.ts(it, 128)]), R(w2s[:]),
                             start=True, stop=False)
            nc.tensor.matmul(pa[:], onesr, b2sr, start=False, stop=True)
            pa_se.append(pa)
            se = P.tile([128, H], F32, tag=f"selfe{it}", name=f"selfe{it}")
            selfe.append(se)
        for jt in range(NT):
            pa = PT.tile([128, H], F32, tag="pt", name=f"pa_n{jt}",
                         padded_shape=[128, 128])
            nc.tensor.matmul(pa[:], R(h1T_n[:, bass.ts(jt, 128)]), R(w2n[:]),
                             start=True, stop=False)
            nc.tensor.matmul(pa[:], onesr, b2nr, start=False, stop=True)
            pa_ne.append(pa)
            ne = P.tile([128, H + 1], BF16, tag=f"nbe{jt}", name=f"nbe{jt}")
            nc.gpsimd.memset(ne[:, H:H + 1], 1.0)
            nbe65.append(ne)

        svb = SM.tile([128, NT], F32, tag="svb")

        # aux copy work, deferred into the jt=0 build stream so in-order
        # engine queues never block on not-yet-ready PSUM producers
        aux = {
            4: lambda: nc.vector.tensor_copy(out=svb[:], in_=psv[:, 0:NT]),
            6: lambda: nc.gpsimd.tensor_scalar_add(out=selfe[0][:],
                                                   in0=pa_se[0][:], scalar1=0.0),
            8: lambda: nc.gpsimd.tensor_scalar_add(out=selfe[1][:],
                                                   in0=pa_se[1][:], scalar1=0.0),
            10: lambda: nc.gpsimd.tensor_scalar_add(out=selfe[2][:],
                                                    in0=pa_se[2][:], scalar1=0.0),
            12: lambda: nc.gpsimd.tensor_scalar_add(out=selfe[3][:],
                                                    in0=pa_se[3][:], scalar1=0.0),
            14: lambda: nc.vector.tensor_copy(out=nbe65[0][:, 0:H], in_=pa_ne[0][:]),
            18: lambda: nc.vector.tensor_copy(out=nbe65[1][:, 0:H], in_=pa_ne[1][:]),
            22: lambda: nc.vector.tensor_copy(out=nbe65[2][:, 0:H], in_=pa_ne[2][:]),
            26: lambda: nc.vector.tensor_copy(out=nbe65[3][:, 0:H], in_=pa_ne[3][:]),
        }

        # ---------- main pass: j-major scores -> exp -> aggregation ----------
        X = [P.tile([128, N], BF16, tag=f"X{jt}", name=f"X{jt}") for jt in range(NT)]
        pagg = [PA.tile([128, H + 1], F32, tag=f"pagg{it}", name=f"pagg{it}",
                        padded_shape=[128, N]) for it in range(NT)]

        def agg(jt):
            for it in range(NT):
                nc.tensor.matmul(pagg[it][:], X[jt][:, bass.ts(it, 128)],
                                 nbe65[jt][:], start=(jt == 0), stop=False)

        n_mm = len(SLOTS)
        # mask/diag bias injects for the first two tiles run during the
        # phase-1 PE idle window
        ps_list = {}
        for jt in range(2):
            ps_list[jt] = PR.tile([128, N], F32, tag="psumR", name=f"ps{jt}")
            nc.tensor.matmul(ps_list[jt][:], idb[:], e30t[jt], start=True,
                             stop=False)
        n_wu2 = int(os.environ.get("GAT_WARMUP2", "0"))
        if n_wu2:
            wu_w2 = SM.tile([H, H], BF16, tag="wu_w2", name="wu_w2")
            nc.vector.memset(wu_w2[:], 0.0)
            wu_ps2 = PT.tile([H, H], F32, tag="pt", name="wu_ps2",
                             padded_shape=[128, 128])
            for _ in range(n_wu2):
                nc.tensor.matmul(wu_ps2[:], wu_w2[:], wu_w2[:], start=True,
                                 stop=True)
        qi = pi = 0
        for jt in range(NT):
            if jt in ps_list:
                ps = ps_list[jt]
            else:
                ps = PR.tile([128, N], F32, tag="psumR", name="ps")
                nc.tensor.matmul(ps[:], idb[:], e30t[jt], start=True, stop=False)
            for k, (kind, j0) in enumerate(SLOTS):
                if jt == 0 and k in aux:
                    aux[k]()
                if jt > 0 and k == 3:
                    # delayed one tile so ACT/PE never stall at the boundary
                    nc.scalar.activation(out=X[jt - 1][:], in_=ps_prev[:],
                                         func=AF.Exp,
                                         bias=svb[:, jt - 1:jt], scale=1.0)
                if jt > 0 and k == 8:
                    agg(jt - 1)
                last = k == n_mm - 1
                if kind == "P":
                    rl = RL.tile([128, N], BF16, tag="rl", name="rl")
                    eng = PAIR_PAT[pi % len(PAIR_PAT)]
                    pi += 1
                    c = jt * 64 + j0 // 2
                    _build(nc, eng, rl[:], Urep[:], V2[:, c:c + 1], AF, OP)
                    nc.tensor.matmul(ps[:], wbf[:, 126 - j0:254 - j0], rl[:],
                                     start=False, stop=last)
                else:
                    rl4 = RL.tile([128, 2, N], FP8, tag="rl4", name="rl4")
                    for q in range(2):
                        eng = QUAD_PAT[qi % len(QUAD_PAT)]
                        qi += 1
                        c = jt * 64 + j0 // 2 + q
                        _build(nc, eng, rl4[:, q, :], Urep[:],
                               V2[:, c:c + 1], AF, OP)
                    nc.tensor.matmul(ps[:], wdr[:, :, 124 - j0:252 - j0], rl4[:],
                                     perf_mode=mybir.MatmulPerfMode.DoubleRow,
                                     start=False, stop=last)
            ps_prev = ps

        # ---------- tail: chunked last exp -> per-chunk agg -> assembly ----
        out_engs = [nc.sync, nc.scalar, nc.gpsimd, nc.sync]
        for it in range(NT):
            nc.scalar.activation(out=X[NT - 1][:, bass.ts(it, 128)],
                                 in_=ps_prev[:, bass.ts(it, 128)], func=AF.Exp,
                                 bias=svb[:, NT - 1:NT], scale=1.0)
            nc.tensor.matmul(pagg[it][:], X[NT - 1][:, bass.ts(it, 128)],
                             nbe65[NT - 1][:], start=False, stop=True)
            den = pagg[it][:, H:H + 1]
            gate = SM.tile([128, 1], F32, tag="gate", name="gate")
            nc.gpsimd.tensor_single_scalar(out=gate[:], in_=den, scalar=1e-6,
                                           op=OP.is_gt)
            dsafe = SM.tile([128, 1], F32, tag="dsafe", name="dsafe")
            nc.gpsimd.tensor_scalar_max(out=dsafe[:], in0=den, scalar1=1e-6)
            recip = SM.tile([128, 1], F32, tag="recip", name="recip")
            nc.vector.reciprocal(out=recip[:], in_=dsafe[:])
            sg = SM.tile([128, H], F32, tag="sg")
            nc.gpsimd.tensor_scalar_mul(out=sg[:], in0=selfe[it][:], scalar1=gate[:])
            ot = SM.tile([128, H], F32, tag="ot")
            nc.gpsimd.scalar_tensor_tensor(out=ot[:], in0=pagg[it][:, 0:H],
                                           scalar=recip[:], in1=sg[:],
                                           op0=OP.mult, op1=OP.add)
            out_engs[it].dma_start(out=t["out"].ap()[bass.ts(it, 128), :], in_=ot[:])


def _build(nc, eng, out_ap, urep_ap, v2col, AF, OP):
    """relu(Urep + V2[:, col]) on the chosen engine."""
    if eng == "v":
        nc.vector.tensor_scalar(out=out_ap, in0=urep_ap, scalar1=v2col,
                                scalar2=0.0, op0=OP.add, op1=OP.max)
    elif eng == "a":
        nc.scalar.activation(out=out_ap, in_=urep_ap, func=AF.Relu,
                             bias=v2col, scale=1.0)
    else:
        nc.gpsimd.tensor_scalar(out=out_ap, in0=urep_ap, scalar1=v2col,
                                scalar2=0.0, op0=OP.add, op1=OP.max)


def _host_constants(inputs):
    f32 = np.float32
    bf = ml_dtypes.bfloat16
    H_ = H
    w2 = np.asarray(inputs["comb_w2"], f32)            # [H, 1]
    w08 = 0.8 * w2[:, 0]
    wdr = np.zeros((128, 2, 252), f32)
    for g in range(2):
        for q in range(2):
            wdr[g * H_:(g + 1) * H_, q, 124 + 2 * q + g] = w08
    wbf = np.zeros((128, 254), f32)
    wbf[0:H_, 126] = w08
    wbf[H_:128, 127] = w08
    wpack1 = np.concatenate([np.asarray(inputs["nb_w1"], f32),
                             np.asarray(inputs["self_w1"], f32)], axis=1)
    bvec = np.stack([
        np.asarray(inputs["self_b1"], f32),
        np.asarray(inputs["nb_b1"], f32),
        np.asarray(inputs["self_b2"], f32),
        np.asarray(inputs["nb_b2"], f32),
        np.asarray(inputs["comb_b1"], f32),
    ], axis=1)
    w1c_s = np.asarray(inputs["comb_w1"], f32)[:H_]     # [H, H]
    w1c_n = np.asarray(inputs["comb_w1"], f32)[H_:]
    w2s_ = np.asarray(inputs["self_w2"], f32)
    w2n_ = np.asarray(inputs["nb_w2"], f32)
    cV = w1c_n.T @ np.asarray(inputs["nb_b2"], f32)
    cU = w1c_s.T @ np.asarray(inputs["self_b2"], f32) + np.asarray(
        inputs["comb_b1"], f32)
    wpack2 = np.concatenate([
        w2n_, w2s_, w2n_ @ w1c_n, w2s_ @ w1c_s,
        cV[:, None], cU[:, None],
    ], axis=1)
    bfpack = np.zeros((H_, 776), f32)
    bfpack[:, 0] = 0.2 * w2[:, 0]
    bfpack[0, 8:520] = 1.0
    bfpack[0, 520:584] = np.asarray(inputs["self_b2"], f32)
    bfpack[0, 584:648] = np.asarray(inputs["nb_b2"], f32)
    bfpack[0, 648:712] = np.asarray(inputs["nb_b1"], f32)
    bfpack[0, 712:776] = np.asarray(inputs["self_b1"], f32)
    consts = {
        "wpack1": np.ascontiguousarray(wpack1),
        "wpack2": np.ascontiguousarray(wpack2),
        "bfpack": bfpack.astype(bf),
        "w2bd_dr": wdr.astype(NP_FP8),
        "w2bd_bf": wbf.astype(bf),
        "id_bf16": np.eye(128, dtype=f32).astype(bf),
    }
    return consts


def _host_percore(inputs):
    """Per-core tensors: transposed nodes + additive mask bias."""
    f32 = np.float32
    bf = ml_dtypes.bfloat16
    nodes = np.asarray(inputs["nodes"], f32).reshape(B, N, D)
    nodes_t = np.ascontiguousarray(nodes.transpose(0, 2, 1))      # [B, D, N]
    mask = (np.asarray(inputs["edges"]) != 0)
    mask &= ~np.eye(N, dtype=bool)[None]
    e30 = np.where(mask, np.float32(0.0), np.float32(-30.0)).astype(bf)
    return nodes_t, e30


def _build_fast_path(nc):
    """Cache a single jitted shard_map executable so repeat kernel() calls
    skip jax re-tracing (same lowering run_bass_kernel_spmd uses under axon)."""
    import jax
    from jax.sharding import Mesh, PartitionSpec
    from jax.experimental.shard_map import shard_map

    bass2jax.install_neuronx_cc_hook()
    pname = nc.partition_id_tensor.name if nc.partition_id_tensor else None
    in_names, out_names, out_avals = [], [], []
    for alloc in nc.m.functions[0].allocations:
        if not isinstance(alloc, mybir.MemoryLocationSet):
            continue
        name = alloc.memorylocations[0].name
        if alloc.kind == "ExternalInput":
            if name != pname:
                in_names.append(name)
        elif alloc.kind == "ExternalOutput":
            out_names.append(name)
            out_avals.append(jax.core.ShapedArray(tuple(alloc.tensor_shape),
                                                  mybir.dt.np(alloc.dtype)))
    all_names = in_names + out_names + ([pname] if pname else [])

    def _body(*args):
        operands = list(args)
        if pname is not None:
            operands.append(bass2jax.partition_id_tensor())
        return tuple(bass2jax._bass_exec_p.bind(
            *operands, out_avals=tuple(out_avals), in_names=tuple(all_names),
            out_names=tuple(out_names), lowering_input_output_aliases=(),
            sim_require_finite=True, sim_require_nnan=True, nc=nc))

    devices = jax.devices()[:NCORES]
    mesh = Mesh(np.asarray(devices), ("core",))
    n_io = len(in_names) + len(out_names)
    sharded = jax.jit(
        shard_map(_body, mesh=mesh, in_specs=(PartitionSpec("core"),) * n_io,
                  out_specs=(PartitionSpec("core"),) * len(out_names),
                  check_rep=False),
        keep_unused=True,
    )
    return sharded, in_names, out_names, out_avals


def kernel(**inputs):
    first = "nc" not in _CACHE
    if first:
        _CACHE["nc"] = _build_module()
    nc = _CACHE["nc"]

    consts = _host_constants(inputs)
    nodes_t, e30 = _host_percore(inputs)

    in_maps = []
    for c in range(NCORES):
        m = dict(consts)
        m["nodes_t"] = nodes_t[c]
        m["e30"] = e30[c]
        in_maps.append(m)

    if first:
        res = run_bass_kernel_spmd(nc, in_maps, core_ids=list(range(NCORES)))
        _CACHE["fast"] = _build_fast_path(nc)
        return np.stack([res.results[c]["out"] for c in range(NCORES)]).astype(np.float32)

    import jax
    sharded, in_names, out_names, out_avals = _CACHE["fast"]
    ckey = hash(tuple((k, v.tobytes()) for k, v in sorted(consts.items())))
    if _CACHE.get("ckey") != ckey:
        _CACHE["cdev"] = {
            n: jax.device_put(np.concatenate([np.asarray(in_maps[c][n])
                                              for c in range(NCORES)], axis=0))
            for n in in_names if n not in ("nodes_t", "e30")
        }
        _CACHE["zdev"] = [jax.device_put(np.zeros((NCORES * a.shape[0], *a.shape[1:]),
                                                  a.dtype)) for a in out_avals]
        _CACHE["ckey"] = ckey
    cdev = _CACHE["cdev"]
    concat_in = [cdev[n] if n in cdev else
                 np.concatenate([np.asarray(in_maps[c][n]) for c in range(NCORES)], axis=0)
                 for n in in_names]
    outs = sharded(*concat_in, *_CACHE["zdev"])
    i = out_names.index("out")
    return np.asarray(outs[i]).reshape(NCORES, N, H).astype(np.float32)
